# revision 1
# baseline (speedup 1.0000x reference)
"""Trainium2 Bass kernel for the GCA sparse-attention module.

Math (per batch b):
    a  = emb_a[word_seq] @ lin_w + lin_b                    # [W, H]
    u  = hidden @ a.T / sqrt(H)                             # [L, W]
    e  = exp(u) * (label > 0)                               # [L, W]
    p  = e / (sum_w e + 1e-10)
    o  = sum_w p * emb_c[label]                             # [L, H]

Structure:
  * u = hidden @ (g @ lin_w + lb).T = (hidden @ lin_w.T_ext) @ g_ext.T
    with lin_b folded in via an extra 1.0 column baked into the padded
    bf16 embedding table on the host; the projection h2xT depends only
    on hidden+weights so the gather stays off the critical path.
  * u is computed TRANSPOSED (uT[w, l]) so the per-label masked sums
    qe[n, l] = sum_w e*[label==n] reduce over the PARTITION axis — done
    on the PE as 10 accumulating matmuls with constant one-hot lhsT
    weights, not on the DVE.  o = qeT.T @ emb_c then needs no transpose.
  * normalizer s = sum_w e*[label>0] via one extra masked product and a
    GpSimd partition-axis reduce; 1/s is applied during the PSUM->SBUF
    output copies (per-partition activation scale).
  * everything streams through the PE in bf16 (1 cyc/col vs 4 for f32).

Sharding: 8 cores = (batch b, L-half) pairs; fully independent.
"""

import numpy as np
import ml_dtypes

import concourse.bass as bass
import concourse.mybir as mybir
import concourse.tile as tile
from concourse import bacc
from concourse import bass_utils
from concourse.masks import make_identity

# Problem shapes (hardcoded per contract).
B, L, W = 4, 512, 256
VOCAB, E, H = 30000, 300, 768
EP = 384                    # E padded: 300 data + 83 zero + 1 ones/bias col
NL = 6
P = 128
NCORES = 8
LC = L * B // NCORES        # 256 l-rows per core
WT = W // P                 # 2 w-tiles
LT = LC // P                # 2 l-tiles
HT = H // P                 # 6 h-tiles
ET = EP // P                # 3 e-chunks
TEMPER = float(H) ** 0.5

F32 = mybir.dt.float32
BF16 = mybir.dt.bfloat16
I32 = mybir.dt.int32
I8 = mybir.dt.int8
BF = ml_dtypes.bfloat16

TRACE = False  # test.py flips this for profiled runs

_CACHE = {}


def _build():
    """Build + compile the per-core Bass program (identical on all cores)."""
    nc = bacc.Bacc("TRN2", debug=False, num_devices=1)

    emb_a = nc.dram_tensor("emb_a", [VOCAB, EP], BF16, kind="ExternalInput").ap()
    widx = nc.dram_tensor("widx", [P, WT], I32, kind="ExternalInput").ap()
    # hT packed on host: hT[p, m*LC + l] = hidden[l, m*128 + p]
    hT_d = nc.dram_tensor("hT", [P, HT * LC], BF16, kind="ExternalInput").ap()
    # labels TRANSPOSED+packed: lab[p, j*LC + l] = label[l, j*128 + p]
    lab_d = nc.dram_tensor("label", [P, WT * LC], I8, kind="ExternalInput").ap()
    # lwx packed on host: lwx[p, m*EP + e] = lin_w.T_ext[m*128 + p, e]
    lwx_d = nc.dram_tensor("lwx", [P, HT * EP], BF16, kind="ExternalInput").ap()
    # emb_c rows 1..5 in rows 0..4 of an 8-row tensor (rows 5..7 zero)
    ec_d = nc.dram_tensor("emb_c", [8, H], BF16, kind="ExternalInput").ap()
    o_d = nc.dram_tensor("o", [LC, H], BF16, kind="ExternalOutput").ap()

    with tile.TileContext(nc) as tc:
        with (
            tc.tile_pool(name="cst", bufs=1) as cst,
            tc.tile_pool(name="sb", bufs=1) as sb,
            tc.tile_pool(name="wrk", bufs=4) as wrk,
            tc.tile_pool(name="psA", bufs=3, space="PSUM") as psA,
            tc.tile_pool(name="psO", bufs=2, space="PSUM") as psO,
            tc.tile_pool(name="psQ", bufs=2, space="PSUM") as psQ,
            tc.tile_pool(name="psR", bufs=1, space="PSUM") as psR,
        ):
            # ---- word indices, then gathers as early as possible ----
            wt = cst.tile([P, WT], I32, name="wt")
            nc.sync.dma_start(out=wt[:], in_=widx)

            g = []
            for j in range(WT):
                t = sb.tile([P, EP], BF16, name=f"g{j}", tag=f"g{j}")
                nc.gpsimd.indirect_dma_start(
                    out=t[:],
                    out_offset=None,
                    in_=emb_a,
                    in_offset=bass.IndirectOffsetOnAxis(ap=wt[:, j : j + 1], axis=0),
                )
                g.append(t)

            # ---- bulk input DMAs: hT on sync, the rest on scalar ----
            hT = sb.tile([P, HT * LC], BF16, name="hT", tag="hT")
            hhalf = HT * LC // 2
            nc.sync.dma_start(out=hT[:, :hhalf], in_=hT_d[:, :hhalf])
            nc.sync.dma_start(out=hT[:, hhalf:], in_=hT_d[:, hhalf:])
            hm = [hT[:, m * LC : (m + 1) * LC] for m in range(HT)]

            lwx = sb.tile([P, HT * EP], BF16, name="lwx", tag="lwx")
            lhalf = HT * EP // 2
            nc.scalar.dma_start(out=lwx[:, :lhalf], in_=lwx_d[:, :lhalf])
            nc.scalar.dma_start(out=lwx[:, lhalf:], in_=lwx_d[:, lhalf:])

            lab = cst.tile([P, WT * LC], I8, name="lab")
            nc.scalar.dma_start(out=lab[:], in_=lab_d)

            ec = cst.tile([8, H], BF16, name="ec")
            nc.scalar.dma_start(out=ec[:], in_=ec_d)

            # ---- label masks in [w, l] layout (ready early) ----
            labf = sb.tile([P, WT * LC], BF16, name="labf", tag="labf")
            nc.vector.tensor_copy(out=labf[:], in_=lab[:])
            mask0 = sb.tile([P, WT * LC], BF16, name="mask0", tag="mask0")
            nc.vector.tensor_scalar(
                out=mask0[:], in0=labf[:],
                scalar1=0.5, scalar2=None,
                op0=mybir.AluOpType.is_gt,
            )
            masks = []
            for n in range(1, NL):
                t = sb.tile([P, WT * LC], BF16, name=f"msk{n}", tag=f"msk{n}")
                nc.vector.tensor_scalar(
                    out=t[:], in0=labf[:],
                    scalar1=float(n), scalar2=None,
                    op0=mybir.AluOpType.is_equal,
                )
                masks.append(t)

            # ---- constants on gpsimd (after gathers): identity, one-hots ----
            ident = cst.tile([P, P], BF16, name="ident")
            make_identity(nc, ident[:])
            ones11 = cst.tile([1, 1], F32, name="ones11")
            nc.gpsimd.memset(ones11[:], 1.0)
            onehot = []
            for n in range(NL - 1):
                t = cst.tile([P, 8], BF16, name=f"oh{n}")
                nc.gpsimd.memset(t[:], 0.0)
                nc.gpsimd.memset(t[:, n : n + 1], 1.0)
                onehot.append(t)

            # ---- h2xT[c] = (hidden @ lwx)ᵀ chunk: [128e, 256l] ----
            pe = [psA.tile([P, LC], F32, name=f"pe{c}", tag="acc") for c in range(ET)]
            for m in range(HT):
                for c in range(ET):
                    nc.tensor.matmul(
                        out=pe[c][:],
                        lhsT=lwx[:, m * EP + c * P : m * EP + (c + 1) * P],
                        rhs=hm[m],
                        start=(m == 0),
                        stop=(m == HT - 1),
                    )
            h2t = []
            for c in range(ET):
                t = sb.tile([P, LC], BF16, name=f"h2t{c}", tag=f"h2t{c}")
                if c == 1:
                    nc.vector.tensor_copy(out=t[:], in_=pe[c][:])
                else:
                    nc.scalar.copy(out=t[:], in_=pe[c][:])
                h2t.append(t)

            # ---- gT[c][e,w] via PE transposes of the gathered rows ----
            gT = []
            for c in range(ET):
                t = sb.tile([P, W], BF16, name=f"gT{c}", tag=f"gT{c}")
                gT.append(t)
            for j in range(WT):
                for c in range(ET):
                    pt = psQ.tile([P, P], BF16, name="pt", tag="pt")
                    nc.tensor.transpose(
                        out=pt[:], in_=g[j][:, c * P : (c + 1) * P], identity=ident[:]
                    )
                    nc.vector.tensor_copy(
                        out=gT[c][:, j * P : (j + 1) * P], in_=pt[:]
                    )

            # ---- uT per w-tile + exp: eT[j] = exp(uT/temper) [128w, 256l] ----
            pu = [psA.tile([P, LC], F32, name=f"pu{j}", tag="acc") for j in range(WT)]
            eT = []
            for j in range(WT):
                for c in range(ET):
                    nc.tensor.matmul(
                        out=pu[j][:],
                        lhsT=gT[c][:, j * P : (j + 1) * P],
                        rhs=h2t[c][:],
                        start=(c == 0),
                        stop=(c == ET - 1),
                    )
            for j in range(WT):
                e = sb.tile([P, LC], BF16, name=f"eT{j}", tag=f"eT{j}")
                nc.scalar.activation(
                    out=e[:], in_=pu[j][:],
                    func=mybir.ActivationFunctionType.Exp,
                    scale=1.0 / TEMPER,
                )
                eT.append(e)

            # ---- normalizer: s[l] = sum_w e*[lab>0], r = 1/(s+eps) ----
            sx = sb.tile([P, WT * LC], BF16, name="sx", tag="sx")
            for j in range(WT):
                nc.vector.tensor_mul(
                    out=sx[:, j * LC : (j + 1) * LC],
                    in0=mask0[:, j * LC : (j + 1) * LC],
                    in1=eT[j][:],
                )
            srow = sb.tile([1, WT * LC], F32, name="srow", tag="srow")
            for j in range(WT):
                nc.gpsimd.tensor_reduce(
                    out=srow[:, j * LC : (j + 1) * LC],
                    in_=sx[:, j * LC : (j + 1) * LC],
                    axis=mybir.AxisListType.C, op=mybir.AluOpType.add,
                )
            ssum = sb.tile([1, LC], F32, name="ssum", tag="ssum")
            nc.vector.tensor_add(
                out=ssum[:], in0=srow[:, :LC], in1=srow[:, LC:]
            )
            rr = []
            for i in range(LT):
                ps_r = psR.tile([P, 1], F32, name="psr", tag="psr")
                nc.tensor.matmul(
                    out=ps_r[:, 0:1],
                    lhsT=ssum[:, i * P : (i + 1) * P],
                    rhs=ones11[:],
                    start=True,
                    stop=True,
                )
                r = sb.tile([P, 1], F32, name=f"r{i}", tag=f"r{i}")
                nc.vector.tensor_scalar_add(out=r[:], in0=ps_r[:, 0:1], scalar1=1e-10)
                nc.vector.reciprocal(out=r[:], in_=r[:])
                rr.append(r)

            # ---- qeT[n, l] via one-hot accumulating matmuls on the PE ----
            pqe = psA.tile([8, LC], F32, name="pqe", tag="acc")
            for n in range(1, NL):
                for j in range(WT):
                    scr = wrk.tile([P, LC], BF16, name="scr", tag="scr")
                    nc.vector.tensor_mul(
                        out=scr[:],
                        in0=masks[n - 1][:, j * LC : (j + 1) * LC],
                        in1=eT[j][:],
                    )
                    nc.tensor.matmul(
                        out=pqe[:],
                        lhsT=onehot[n - 1][:],
                        rhs=scr[:],
                        start=(n == 1 and j == 0),
                        stop=(n == NL - 1 and j == WT - 1),
                    )
            qeS = sb.tile([8, LC], BF16, name="qeS", tag="qeS")
            nc.scalar.copy(out=qeS[:], in_=pqe[:])

            # ---- output: o[l,:] = r[l] * (qeT[:,l] . emb_c[1:6]) ----
            for i in range(LT):
                o = sb.tile([P, H], BF16, name=f"o{i}", tag=f"o{i}")
                for half in range(2):
                    po = psO.tile([P, H // 2], F32, name="po", tag="po")
                    nc.tensor.matmul(
                        out=po[:],
                        lhsT=qeS[:, i * P : (i + 1) * P],
                        rhs=ec[:, half * (H // 2) : (half + 1) * (H // 2)],
                        start=True,
                        stop=True,
                    )
                    if (i + half) % 2 == 0:
                        nc.scalar.activation(
                            out=o[:, half * (H // 2) : (half + 1) * (H // 2)],
                            in_=po[:],
                            func=mybir.ActivationFunctionType.Copy,
                            bias=0.0, scale=rr[i][:, 0:1],
                        )
                    else:
                        nc.vector.tensor_scalar(
                            out=o[:, half * (H // 2) : (half + 1) * (H // 2)],
                            in0=po[:],
                            scalar1=rr[i][:, 0:1], scalar2=None,
                            op0=mybir.AluOpType.mult,
                        )
                nc.sync.dma_start(out=o_d[i * P : (i + 1) * P, :], in_=o[:])

    nc.compile()
    return nc


def _get_nc():
    if "nc" not in _CACHE:
        _CACHE["nc"] = _build()
    return _CACHE["nc"]


def _prep_shared(inputs):
    """Host-side packing shared across cores."""
    ea = np.asarray(inputs["emb_a"], dtype=np.float32)
    lw = np.asarray(inputs["lin_w"], dtype=np.float32)
    lb = np.asarray(inputs["lin_b"], dtype=np.float32)
    ec = np.asarray(inputs["emb_c"], dtype=np.float32)

    # padded bf16 embedding table; last column = 1.0 feeds the bias term
    ea_p = np.zeros((VOCAB, EP), dtype=BF)
    ea_p[:, :E] = ea.astype(BF)
    ea_p[:, EP - 1] = BF(1.0)
    # lin_w.T extended with lin_b in the matching last column, then packed
    lwx = np.zeros((H, EP), dtype=BF)
    lwx[:, :E] = lw.T.astype(BF)
    lwx[:, EP - 1] = lb.astype(BF)
    lwx_p = np.ascontiguousarray(
        lwx.reshape(HT, P, EP).transpose(1, 0, 2).reshape(P, HT * EP)
    )
    ec8 = np.zeros((8, H), dtype=BF)
    ec8[: NL - 1] = ec[1:].astype(BF)
    return ea_p, lwx_p, ec8


def _core_map(inputs, ea_p, lwx_p, ec8, core):
    ws = np.asarray(inputs["word_seq"]).astype(np.int32)
    hs = np.asarray(inputs["hidden_state"], dtype=np.float32)
    lvm = np.asarray(inputs["label_value_matrix"]).astype(np.int8)
    b, half = divmod(core, 2)
    lsl = slice(half * LC, (half + 1) * LC)
    hT = hs[b, lsl].T.astype(BF)  # [H, LC]
    hT_p = np.ascontiguousarray(
        hT.reshape(HT, P, LC).transpose(1, 0, 2).reshape(P, HT * LC)
    )
    labT = lvm[b, lsl].T  # [W, LC] int8
    labT_p = np.ascontiguousarray(
        labT.reshape(WT, P, LC).transpose(1, 0, 2).reshape(P, WT * LC)
    )
    return {
        "emb_a": ea_p,
        "widx": np.ascontiguousarray(ws[b].reshape(WT, P).T),
        "hT": hT_p,
        "label": labT_p,
        "lwx": lwx_p,
        "emb_c": ec8,
    }


def kernel(**inputs):
    nc = _get_nc()
    ea_p, lwx_p, ec8 = _prep_shared(inputs)
    in_maps = [_core_map(inputs, ea_p, lwx_p, ec8, c) for c in range(NCORES)]

    res = bass_utils.run_bass_kernel_spmd(
        nc, in_maps, core_ids=list(range(NCORES)), trace=TRACE
    )
    _CACHE["last_result"] = res

    out = np.empty((B, L, H), np.float32)
    for c in range(NCORES):
        b, half = divmod(c, 2)
        out[b, half * LC : (half + 1) * LC] = np.asarray(
            res.results[c]["o"]
        ).astype(np.float32)
    return out



# revision 8
# speedup vs baseline: 2.9640x; 2.9640x over previous
"""Trainium2 Bass kernel for the GCA sparse-attention module.

Math (per batch b):
    a  = emb_a[word_seq] @ lin_w + lin_b                    # [W, H]
    u  = hidden @ a.T / sqrt(H)                             # [L, W]
    e  = exp(u) * (label > 0)                               # [L, W]
    p  = e / (sum_w e + 1e-10)
    o  = sum_w p * emb_c[label]                             # [L, H]

Structure:
  * u = hidden @ (g @ lin_w + lb).T = (hidden @ lin_w.T_ext) @ g_ext.T
    with lin_b folded in via an extra 1.0 column in the padded gathered
    rows; the embedding lookup + transpose is host-side packing so the
    device sees plain dense DMAs (no indirect gather, no PE transposes).
  * u is computed TRANSPOSED (uT[w, l]) so the per-label masked sums
    qe[n, l] = sum_w e*[label==n] reduce over the PARTITION axis — done
    on the PE as 10 accumulating matmuls with constant one-hot lhsT
    weights.  Each one-hot column ALSO carries a 1 in column 0, so PSUM
    row 0 accumulates the normalizer s[l] = sum_w e*[label>0] for free.
  * r = 1/(s+eps) lives along the FREE axis of qe, so it is applied by
    one tiny replicate-matmul + one [8,256] multiply (no partition-axis
    reduce, no gpsimd, no transposes of s).
  * o = (r*qe)T.T @ emb_c needs no transpose; everything streams through
    the PE in bf16 (1 cyc/col vs 4 for f32).

Sharding: 8 cores = (batch b, L-half) pairs; fully independent.
"""

import numpy as np
import ml_dtypes

import concourse.bass as bass
import concourse.mybir as mybir
import concourse.tile as tile
from concourse import bacc
from concourse import bass_utils

# Problem shapes (hardcoded per contract).
B, L, W = 4, 512, 256
VOCAB, E, H = 30000, 300, 768
EP = 384                    # E padded: 300 data + 83 zero + 1 ones/bias col
NL = 6
P = 128
NCORES = 8
LC = L * B // NCORES        # 256 l-rows per core
WT = W // P                 # 2 w-tiles
LT = LC // P                # 2 l-tiles
HT = H // P                 # 6 h-tiles
ET = EP // P                # 3 e-chunks
TEMPER = float(H) ** 0.5

F32 = mybir.dt.float32
BF16 = mybir.dt.bfloat16
I8 = mybir.dt.int8
BF = ml_dtypes.bfloat16

TRACE = False  # test.py flips this for profiled runs

_CACHE = {}


def _build():
    """Build + compile the per-core Bass program (identical on all cores)."""
    nc = bacc.Bacc("TRN2", debug=False, num_devices=1)

    # hT packed on host: hT[p, m*LC + l] = hidden[l, m*128 + p]
    hT_d = nc.dram_tensor("hT", [P, HT * LC], BF16, kind="ExternalInput").ap()
    # lwx packed on host: lwx[p, m*EP + e] = lin_w.T_ext[m*128 + p, e]
    lwx_d = nc.dram_tensor("lwx", [P, HT * EP], BF16, kind="ExternalInput").ap()
    # gathered+extended+transposed rows: gT[p, c*W + w] = g_ext[w, c*128 + p]
    gT_d = nc.dram_tensor("gT", [P, ET * W], BF16, kind="ExternalInput").ap()
    # labels TRANSPOSED+packed: lab[p, j*LC + l] = label[l, j*128 + p]
    lab_d = nc.dram_tensor("label", [P, WT * LC], I8, kind="ExternalInput").ap()
    # emb_c rows 1..5 in rows 1..5 of an 8-row tensor (rows 0,6,7 zero)
    ec_d = nc.dram_tensor("emb_c", [8, H], BF16, kind="ExternalInput").ap()
    # constants: col 8n+(n+1) = one-hot for label n+1, col 8n+0 = 1
    # (normalizer accumulates in pqe row 0); row 0 of cols 40..47 = ones
    oh_d = nc.dram_tensor("oh", [P, 48], BF16, kind="ExternalInput").ap()
    o_d = nc.dram_tensor("o", [LC, H], BF16, kind="ExternalOutput").ap()

    with tile.TileContext(nc) as tc:
        with (
            tc.tile_pool(name="cst", bufs=1) as cst,
            tc.tile_pool(name="sb", bufs=1) as sb,
            tc.tile_pool(name="wrk", bufs=4) as wrk,
            tc.tile_pool(name="psA", bufs=3, space="PSUM") as psA,
            tc.tile_pool(name="psO", bufs=2, space="PSUM") as psO,
            tc.tile_pool(name="psR", bufs=1, space="PSUM") as psR,
        ):
            # ---- input DMAs, chunked so compute can start on chunk 0 ----
            # scalar HWDGE ring: small constants, then lwx h-chunks
            ec = cst.tile([8, H], BF16, name="ec")
            nc.scalar.dma_start(out=ec[:], in_=ec_d)
            oh = cst.tile([P, 48], BF16, name="oh")
            nc.scalar.dma_start(out=oh[:], in_=oh_d)
            lwxm = []
            for m in range(HT):
                t = sb.tile([P, EP], BF16, name=f"lwx{m}", tag=f"lwx{m}")
                nc.scalar.dma_start(out=t[:], in_=lwx_d[:, m * EP : (m + 1) * EP])
                lwxm.append(t)
            # sync HWDGE ring: labels, hT h-chunks, gathered rows
            lab = cst.tile([P, WT * LC], I8, name="lab")
            nc.sync.dma_start(out=lab[:], in_=lab_d)
            hm = []
            for m in range(HT):
                t = sb.tile([P, LC], BF16, name=f"hT{m}", tag=f"hT{m}")
                nc.sync.dma_start(out=t[:], in_=hT_d[:, m * LC : (m + 1) * LC])
                hm.append(t)
            gT = []
            for c in range(ET):
                t = sb.tile([P, W], BF16, name=f"gT{c}", tag=f"gT{c}")
                nc.sync.dma_start(out=t[:], in_=gT_d[:, c * W : (c + 1) * W])
                gT.append(t)

            # ---- label masks in [w, l] layout (DVE, overlaps h2t) ----
            labf = sb.tile([P, WT * LC], BF16, name="labf", tag="labf")
            nc.vector.tensor_copy(out=labf[:], in_=lab[:])
            masks = []
            for n in range(1, NL):
                t = sb.tile([P, WT * LC], BF16, name=f"msk{n}", tag=f"msk{n}")
                nc.vector.tensor_scalar(
                    out=t[:], in0=labf[:],
                    scalar1=float(n), scalar2=None,
                    op0=mybir.AluOpType.is_equal,
                )
                masks.append(t)

            # ---- h2xT[c] = (hidden @ lwx)ᵀ chunk: [128e, 256l] ----
            pe = [psA.tile([P, LC], F32, name=f"pe{c}", tag="acc") for c in range(ET)]
            for m in range(HT):
                for c in range(ET):
                    nc.tensor.matmul(
                        out=pe[c][:],
                        lhsT=lwxm[m][:, c * P : (c + 1) * P],
                        rhs=hm[m][:],
                        start=(m == 0),
                        stop=(m == HT - 1),
                    )
            h2t = []
            for c in range(ET):
                t = sb.tile([P, LC], BF16, name=f"h2t{c}", tag=f"h2t{c}")
                if c == 1:
                    nc.scalar.copy(out=t[:], in_=pe[c][:])
                else:
                    nc.vector.tensor_copy(out=t[:], in_=pe[c][:])
                h2t.append(t)

            # ---- uT per w-tile + exp: eT[j] = exp(uT/temper) [128w, 256l] ----
            pu = [psA.tile([P, LC], F32, name=f"pu{j}", tag="acc") for j in range(WT)]
            for j in range(WT):
                for c in range(ET):
                    nc.tensor.matmul(
                        out=pu[j][:],
                        lhsT=gT[c][:, j * P : (j + 1) * P],
                        rhs=h2t[c][:],
                        start=(c == 0),
                        stop=(c == ET - 1),
                    )
            eT = []
            for j in range(WT):
                e = sb.tile([P, LC], BF16, name=f"eT{j}", tag=f"eT{j}")
                nc.scalar.activation(
                    out=e[:], in_=pu[j][:],
                    func=mybir.ActivationFunctionType.Exp,
                    scale=1.0 / TEMPER,
                )
                eT.append(e)

            # ---- qeT[n, l] rows 1..5 + normalizer row 0, all on the PE ----
            pqe = psA.tile([8, LC], F32, name="pqe", tag="acc")
            k = 0
            for n in range(1, NL):
                for j in range(WT):
                    scr = wrk.tile([P, LC], BF16, name="scr", tag="scr")
                    nc.vector.tensor_mul(
                        out=scr[:],
                        in0=masks[n - 1][:, j * LC : (j + 1) * LC],
                        in1=eT[j][:],
                    )
                    nc.tensor.matmul(
                        out=pqe[:],
                        lhsT=oh[:, (n - 1) * 8 : n * 8],
                        rhs=scr[:],
                        start=(k == 0),
                        stop=(k == (NL - 1) * WT - 1),
                    )
                    k += 1

            # ---- r[l] = 1/(s+eps) along free axis; scale qe rows ----
            qeS = sb.tile([8, LC], BF16, name="qeS", tag="qeS")
            nc.scalar.copy(out=qeS[:], in_=pqe[:])
            rrow = sb.tile([1, LC], F32, name="rrow", tag="rrow")
            nc.vector.tensor_scalar_add(out=rrow[:], in0=pqe[0:1, :], scalar1=1e-10)
            rrbf = sb.tile([1, LC], BF16, name="rrbf", tag="rrbf")
            with nc.allow_low_precision(reason="softmax scale row, tol 2e-2"):
                nc.vector.reciprocal(out=rrbf[:], in_=rrow[:])
            prr = psR.tile([8, LC], F32, name="prr", tag="prr")
            nc.tensor.matmul(
                out=prr[:], lhsT=oh[0:1, 40:48], rhs=rrbf[:], start=True, stop=True
            )
            qeR = sb.tile([8, LC], BF16, name="qeR", tag="qeR")
            nc.vector.tensor_mul(out=qeR[:], in0=qeS[:], in1=prr[:])

            # ---- output: o[l,:] = (r*qe)[:, l] . emb_c[1:6] ----
            for i in range(LT):
                o = sb.tile([P, H], BF16, name=f"o{i}", tag=f"o{i}")
                for half in range(2):
                    po = psO.tile([P, H // 2], F32, name="po", tag="po")
                    nc.tensor.matmul(
                        out=po[:],
                        lhsT=qeR[:, i * P : (i + 1) * P],
                        rhs=ec[:, half * (H // 2) : (half + 1) * (H // 2)],
                        start=True,
                        stop=True,
                    )
                    if (i + half) % 2 == 0:
                        nc.scalar.copy(
                            out=o[:, half * (H // 2) : (half + 1) * (H // 2)],
                            in_=po[:],
                        )
                    else:
                        nc.vector.tensor_copy(
                            out=o[:, half * (H // 2) : (half + 1) * (H // 2)],
                            in_=po[:],
                        )
                nc.sync.dma_start(out=o_d[i * P : (i + 1) * P, :], in_=o[:])

    nc.compile()
    return nc


def _get_nc():
    if "nc" not in _CACHE:
        _CACHE["nc"] = _build()
    return _CACHE["nc"]


def _prep_shared(inputs):
    """Host-side packing shared across cores."""
    ea = np.asarray(inputs["emb_a"], dtype=np.float32)
    lw = np.asarray(inputs["lin_w"], dtype=np.float32)
    lb = np.asarray(inputs["lin_b"], dtype=np.float32)
    ec = np.asarray(inputs["emb_c"], dtype=np.float32)
    ws = np.asarray(inputs["word_seq"]).astype(np.int64)

    # lin_w.T extended with lin_b in the matching last column, then packed
    lwx = np.zeros((H, EP), dtype=BF)
    lwx[:, :E] = lw.T.astype(BF)
    lwx[:, EP - 1] = lb.astype(BF)
    lwx_p = np.ascontiguousarray(
        lwx.reshape(HT, P, EP).transpose(1, 0, 2).reshape(P, HT * EP)
    )
    ec8 = np.zeros((8, H), dtype=BF)
    ec8[1:NL] = ec[1:].astype(BF)  # row 0 is the normalizer slot -> zero

    # extended one-hots + replicate-ones constants
    oh = np.zeros((P, 48), dtype=BF)
    for n0 in range(NL - 1):
        oh[:, 8 * n0 + (n0 + 1)] = BF(1.0)  # label n0+1 -> pqe row n0+1
        oh[:, 8 * n0 + 0] = BF(1.0)         # normalizer -> pqe row 0
    oh[0, 40:48] = BF(1.0)

    # per-batch gathered rows, extended with the bias 1.0 column, transposed
    gT_pb = []
    for b in range(B):
        g_ext = np.zeros((W, EP), dtype=BF)
        g_ext[:, :E] = ea[ws[b]].astype(BF)
        g_ext[:, EP - 1] = BF(1.0)
        gT = np.ascontiguousarray(g_ext.T)  # [EP, W]
        gT_pb.append(
            np.ascontiguousarray(
                gT.reshape(ET, P, W).transpose(1, 0, 2).reshape(P, ET * W)
            )
        )
    return lwx_p, ec8, oh, gT_pb


def _core_map(inputs, lwx_p, ec8, oh, gT_pb, core):
    hs = np.asarray(inputs["hidden_state"], dtype=np.float32)
    lvm = np.asarray(inputs["label_value_matrix"]).astype(np.int8)
    b, half = divmod(core, 2)
    lsl = slice(half * LC, (half + 1) * LC)
    hT = hs[b, lsl].T.astype(BF)  # [H, LC]
    hT_p = np.ascontiguousarray(
        hT.reshape(HT, P, LC).transpose(1, 0, 2).reshape(P, HT * LC)
    )
    labT = lvm[b, lsl].T  # [W, LC] int8
    labT_p = np.ascontiguousarray(
        labT.reshape(WT, P, LC).transpose(1, 0, 2).reshape(P, WT * LC)
    )
    return {
        "hT": hT_p,
        "lwx": lwx_p,
        "gT": gT_pb[b],
        "label": labT_p,
        "emb_c": ec8,
        "oh": oh,
    }


def kernel(**inputs):
    nc = _get_nc()
    lwx_p, ec8, oh, gT_pb = _prep_shared(inputs)
    in_maps = [_core_map(inputs, lwx_p, ec8, oh, gT_pb, c) for c in range(NCORES)]

    res = bass_utils.run_bass_kernel_spmd(
        nc, in_maps, core_ids=list(range(NCORES)), trace=TRACE
    )
    _CACHE["last_result"] = res

    out = np.empty((B, L, H), np.float32)
    for c in range(NCORES):
        b, half = divmod(c, 2)
        out[b, half * LC : (half + 1) * LC] = np.asarray(
            res.results[c]["o"]
        ).astype(np.float32)
    return out
